# revision 1
# baseline (speedup 1.0000x reference)
"""Trainium2 Bass kernel for nn_MemoryOnGpu (retrieval_knn).

Per (query q, dataset d, bucket n): pick b* = argmax_b <q, key_db[b*128+n]>
(b in [0,256)), emit key/value rows b**128+n.  One dataset per core.

Per-core pipeline (vs the fp32 baseline):
  PE   : 2-pass bf16 hi/lo matmul -- pass A qh*kh (K=64), pass B
         ql*kh + qh*kl in one K=128 matmul (hi/lo halves stacked on
         partitions with swapped pairing), accumulated in PSUM.  ~3e-5
         score error vs fp32; half the PE time of a 4-pass fp32 matmul.
  ACT  : stages each 2048-col score batch PSUM -> SBUF (the scalar
         engine is otherwise idle; its per-instruction access bubble
         amortizes over the big copy).
  DVE  : exactly two passes per batch -- a per-bucket tensor_reduce max
         (paired: one instruction covers two batches / 16 buckets), then
         ONE max_index instruction per batch that returns the exact
         positions of all 8 bucket maxima in the 2048-col batch.
  gather: kv rows are pre-permuted bucket-major on the host
         (kv2[n*256+b] = kv[b*128+n]) so the gather offset is just
         batch_base + max_index position (single int add, no decode).
         Gathers use the canonical one-offset-per-partition indirect
         DMA (the only form this NRT executes correctly -- batched
         multi-offset gathers were probed and are broken), 8 per batch,
         spread across the whole chunk so the GPSIMD queue streams.
  HWDGE: one 2KB-per-partition contiguous okv write per batch, fp16.
Host casts the fp16 okv back to fp32 and splits keys/values.

GPSIMD cannot run tensor ops on this image (the compiler's per-engine
ISA check rejects them), so all elementwise/reduction work lives on
DVE/ACT; GPSIMD only generates SWDGE gather descriptors.
"""

import sys

import numpy as np

for _p in ("/opt/trn_rl_repo", "/root/.axon_site/_ro/trn_rl_repo"):
    if _p not in sys.path:
        sys.path.insert(0, _p)

NUM_QUERIES = 1024
NUM_DATASETS = 8
DB_SIZE = 32768
KEY_FEATURES = 64
VALUE_FEATURES = 64
NUM_NEIGHBORS = 128   # == num_buckets == n axis
BS = DB_SIZE // NUM_NEIGHBORS  # 256, argmax range
KVW = 128             # interleaved kv row width (64 key + 64 value)

_NC_CACHE = {}

# tuning knobs
TB = 4          # score tiles (512 cols) per PSUM batch
DVE_RED_EVERY = 5   # every k-th batch reduce on DVE instead of Pool tree
GATHERS_PER_CHUNK = 4   # split the 128-offset gather into this many instrs


def build_nc(Q=NUM_QUERIES, DB=DB_SIZE):
    import concourse.bass as bass
    import concourse.mybir as mybir
    import concourse.tile as tile
    from concourse import bacc

    F = KEY_FEATURES
    NB = NUM_NEIGHBORS
    QC = Q // 128                 # 8 q-chunks
    BCOLS = 512 * TB              # 2048 score cols per batch
    NBATCH = DB // BCOLS          # 16 batches per chunk
    NPB = BCOLS // BS             # buckets per batch = 8
    assert NPB == 8               # max8/max_index operate on exactly 8
    assert GATHERS_PER_CHUNK in (1, 2, 4, 8)

    nc = bacc.Bacc()
    i32 = mybir.dt.int32
    u32 = mybir.dt.uint32
    qT = nc.declare_dram_parameter("qT", [128, Q], mybir.dt.bfloat16, isOutput=False)
    qT2 = nc.declare_dram_parameter("qT2", [64, Q], mybir.dt.bfloat16, isOutput=False)
    kT = nc.declare_dram_parameter("kT", [128, DB], mybir.dt.bfloat16, isOutput=False)
    kv = nc.declare_dram_parameter("kv", [DB, KVW], mybir.dt.float16, isOutput=False)
    consts = nc.declare_dram_parameter("consts", [128, 16], i32, isOutput=False)
    okv = nc.declare_dram_parameter("okv", [Q, NB, KVW], mybir.dt.float16, isOutput=True)

    X = mybir.AxisListType.X
    OP = mybir.AluOpType

    with tile.TileContext(nc) as tc:
        with (
            tc.tile_pool(name="const", bufs=1) as constp,
            tc.tile_pool(name="ps", bufs=2, space="PSUM") as psp,
            tc.tile_pool(name="sc", bufs=3) as scp,
            tc.tile_pool(name="mx", bufs=4) as mxp,
            tc.tile_pool(name="off", bufs=4) as offp,
            tc.tile_pool(name="okv", bufs=4) as okvp,
        ):
            cst = constp.tile([128, 16], i32, tag="cst")
            nc.sync.dma_start(out=cst[:], in_=consts[:])
            qt2 = constp.tile([64, Q], mybir.dt.bfloat16, tag="qt2")
            nc.sync.dma_start(out=qt2[:], in_=qT2[:])
            qt = constp.tile([128, Q], mybir.dt.bfloat16, tag="qt")
            nc.sync.dma_start(out=qt[:], in_=qT[:])
            # one kt tile per batch so the first matmuls only wait on the
            # first slice DMA (tile-granular read deps), split on 2 queues
            kts = []
            for kc in range(NBATCH):
                ktc = constp.tile([128, BCOLS], mybir.dt.bfloat16, tag=f"kt{kc}")
                eng = nc.scalar if kc % 2 == 0 else nc.sync
                eng.dma_start(out=ktc[:],
                              in_=kT[:, kc * BCOLS:(kc + 1) * BCOLS])
                kts.append(ktc)

            for qc in range(QC):
                qa = qt2[:, qc * 128:(qc + 1) * 128]       # qh (partitions 0-63)
                qb = qt[:, qc * 128:(qc + 1) * 128]        # [ql; qh]
                # batches are reduced in pairs (one tensor_reduce per 4096
                # cols) except the first two of the program, which run
                # singly so the DVE pipeline fills ~8us earlier
                if qc == 0:
                    groups = [[0], [1]] + [[b, b + 1] for b in range(2, NBATCH, 2)]
                else:
                    groups = [[b, b + 1] for b in range(0, NBATCH, 2)]
                for grp in groups:
                    sc4 = scp.tile([128, len(grp) * BCOLS],
                                   mybir.dt.float32, tag=f"sc{len(grp)}")
                    for gi, bt in enumerate(grp):
                        ps = psp.tile([128, BCOLS], mybir.dt.float32, tag="ps")
                        for t in range(TB):
                            nc.tensor.matmul(
                                ps[:, t * 512:(t + 1) * 512],
                                qa,
                                kts[bt][0:64, t * 512:(t + 1) * 512],
                                start=True,
                                stop=False,
                            )
                        for t in range(TB):
                            nc.tensor.matmul(
                                ps[:, t * 512:(t + 1) * 512],
                                qb,
                                kts[bt][:, t * 512:(t + 1) * 512],
                                start=False,
                                stop=True,
                            )
                        # stage scores PSUM -> SBUF on the otherwise-idle ACT
                        nc.scalar.copy(
                            out=sc4[:, gi * BCOLS:(gi + 1) * BCOLS],
                            in_=ps[:],
                        )
                    m16 = mxp.tile([128, len(grp) * NPB],
                                   mybir.dt.float32, tag=f"m{len(grp)}")
                    nc.vector.tensor_reduce(
                        m16[:], sc4[:].rearrange("p (n b) -> p n b", b=BS),
                        axis=X, op=OP.max,
                    )
                    for gi, bt in enumerate(grp):
                        # exact positions of the 8 bucket maxes within the
                        # 2048-col batch
                        pidx = mxp.tile([128, NPB], u32, tag="pidx")
                        nc.vector.max_index(
                            out=pidx[:],
                            in_max=m16[:, gi * NPB:(gi + 1) * NPB],
                            in_values=sc4[:, gi * BCOLS:(gi + 1) * BCOLS],
                        )
                        # kv2 is bucket-major on the host, so the gather row
                        # is batch_base + max_index position; the base rides
                        # in the descriptor's constant element_offset, so
                        # pidx feeds the gather with no decode at all
                        n0 = bt * NPB
                        og = okvp.tile([128, NPB * KVW], mybir.dt.float16,
                                       tag="og")
                        ogv = og[:].rearrange("p (n w) -> p n w", w=KVW)
                        for j in range(NPB):
                            nc.gpsimd.indirect_dma_start(
                                out=ogv[:, j, :],
                                out_offset=None,
                                in_=kv[:],
                                in_offset=bass.IndirectOffsetOnAxis(
                                    ap=pidx[:, j:j + 1], axis=0
                                ),
                                element_offset=bt * BCOLS * KVW,
                            )
                        nc.sync.dma_start(
                            out=okv[qc * 128:(qc + 1) * 128, n0:n0 + NPB, :],
                            in_=ogv,
                        )
    if not nc.is_finalized():
        nc.finalize()
    return nc


def _get_nc(Q, DB):
    key = (Q, DB)
    if key not in _NC_CACHE:
        _NC_CACHE[key] = build_nc(Q, DB)
    return _NC_CACHE[key]


def make_core_inputs(query, key_db, value_db, d, Q=NUM_QUERIES, DB=DB_SIZE):
    """Host-side prep of one core's input arrays (dataset d)."""
    import ml_dtypes

    F = KEY_FEATURES
    NB = NUM_NEIGHBORS
    bf16 = ml_dtypes.bfloat16

    q = query[:, d, :].astype(np.float32)                 # (Q, F)
    qh = q.astype(bf16)
    ql = (q - qh.astype(np.float32)).astype(bf16)
    qtile = np.empty((128, Q), dtype=bf16)
    qtile[0:64] = ql.T
    qtile[64:128] = qh.T

    k = key_db[d].astype(np.float32)                      # (DB, F)
    # col = n*BS + b  <->  key row b*NB + n  (bucket-inner column order)
    kperm = k.reshape(BS, NB, F).transpose(2, 1, 0).reshape(F, DB)
    kh = kperm.astype(bf16)
    kl = (kperm - kh.astype(np.float32)).astype(bf16)
    ktile = np.empty((128, DB), dtype=bf16)
    ktile[0:64] = kh
    ktile[64:128] = kl

    kvh = np.concatenate([key_db[d], value_db[d]], axis=1).astype(np.float16)
    # bucket-major reorder: kv2[n*BS + b] = kv[b*NB + n] so a gather offset
    # is batch_base + within-batch position straight from max_index
    kv2 = np.ascontiguousarray(
        kvh.reshape(BS, NB, KVW).transpose(1, 0, 2).reshape(DB, KVW))

    cst = np.broadcast_to(
        (np.arange(16, dtype=np.int32) * 2048)[None, :], (128, 16)
    ).copy()
    return {"qT": qtile, "qT2": np.ascontiguousarray(qh.T), "kT": ktile,
            "kv": kv2, "consts": cst}


def kernel(query, key_db, value_db, num_neighbors):
    from concourse.bass_utils import run_bass_kernel_spmd

    query = np.asarray(query, dtype=np.float32)
    key_db = np.asarray(key_db, dtype=np.float32)
    value_db = np.asarray(value_db, dtype=np.float32)
    assert int(num_neighbors) == NUM_NEIGHBORS
    Q, D, F = query.shape
    _, DB, _ = key_db.shape
    assert (Q, D, F, DB) == (NUM_QUERIES, NUM_DATASETS, KEY_FEATURES, DB_SIZE)

    nc = _get_nc(Q, DB)
    in_maps = [make_core_inputs(query, key_db, value_db, d, Q, DB) for d in range(D)]
    res = run_bass_kernel_spmd(nc, in_maps, core_ids=list(range(D)))

    sel_k = np.empty((Q, D, NUM_NEIGHBORS, KEY_FEATURES), dtype=np.float32)
    sel_v = np.empty((Q, D, NUM_NEIGHBORS, VALUE_FEATURES), dtype=np.float32)
    for d in range(D):
        okv = res.results[d]["okv"]                      # (Q, NB, 128) fp16
        sel_k[:, d] = okv[:, :, :KEY_FEATURES].astype(np.float32)
        sel_v[:, d] = okv[:, :, KEY_FEATURES:].astype(np.float32)
    return sel_k, sel_v

